# revision 18
# baseline (speedup 1.0000x reference)
"""Trainium2 Bass kernel for MHA with RoPE (dense transformer block).

Problem shapes: h [1, 4096, 1024], 16 heads x 64 dim, full (non-causal)
softmax attention, post-softmax all-ones mask (identity), torch-Linear
projections with bias.

Sharding: head-parallel across 8 cores (2 heads/core). Each core:
  - reads full hT (pre-transposed on host, [1024, 4096])
  - computes qT/kT/vT for its 2 heads (feature-major [128, S])
  - RoPE on qT/kT (sign-folded sin table, partition-crossed muls)
  - per 1024-query chunk: scoresT = kT-tiles x qT (MM_DT matmuls),
    exp on ACT (scale=1/8 fused, no max subtraction: |scores| <= ~7),
    PV with a ones-row appended to v (denominator for free),
    normalize with reciprocal broadcast via K=1 matmul,
  - o_proj partial [S, 1024] with its 128 wo columns.
Host sums the 8 partials and adds bo.

All matmul operands are MM_DT (float32r by default — full-rate on the PE
at N>=256; bf16 fallback). The BIR verifier requires fp32r operands to be
produced rounded, so every matmul input tile is MM_DT-typed and written
by a casting producer (gpsimd DMA-cast load, DVE out-cast, ACT out-cast).
"""

import numpy as np

HIDDEN = 1024
HEADS = 16
HEAD_DIM = 64
SEQ = 4096
NCORES = 8
FPC = 128  # features per core = 2 heads x 64

_NC_CACHE = {}


def _build_nc(S=SEQ, mm_dt="float32r"):
    import concourse.bass as bass
    import concourse.tile as tile
    from concourse import mybir
    from concourse.masks import make_identity
    from contextlib import ExitStack

    f32 = mybir.dt.float32
    MM = getattr(mybir.dt, mm_dt)
    Exp = mybir.ActivationFunctionType.Exp

    D = HEAD_DIM
    HID = HIDDEN
    KT = HID // 128          # hidden contraction tiles
    PC = 512                 # projection seq chunk
    NPC = S // PC
    CH = min(1024, S)        # attention query chunk
    HF = CH // 2             # psum half-chunk
    NCH = S // CH
    SK = S // 128            # key tiles

    nc = bass.Bass(trn_type="TRN2")

    # matmul-facing inputs are pre-converted to MM dtype on the host so the
    # loads go over fast HWDGE queues with no cast
    hT = nc.dram_tensor("hT", [HID, S], MM, kind="ExternalInput")
    wqT = nc.dram_tensor("wqT", [HID, FPC], MM, kind="ExternalInput")
    wkT = nc.dram_tensor("wkT", [HID, FPC], MM, kind="ExternalInput")
    wvT = nc.dram_tensor("wvT", [HID, FPC], MM, kind="ExternalInput")
    bqkv = nc.dram_tensor("bqkv", [FPC, 3], f32, kind="ExternalInput")
    woT = nc.dram_tensor("woT", [FPC, HID], MM, kind="ExternalInput")
    cosT = nc.dram_tensor("cosT", [D, S], f32, kind="ExternalInput")
    sinTs = nc.dram_tensor("sinTs", [D, S], f32, kind="ExternalInput")
    out = nc.dram_tensor("out", [S, HID], f32, kind="ExternalOutput")

    hT3 = hT[:, :].rearrange("(ko p) s -> p ko s", p=128)

    with tile.TileContext(nc) as tc, ExitStack() as top:
        sing = top.enter_context(tc.tile_pool(name="sing", bufs=1))

        wq_sb = sing.tile([128, KT, FPC], MM)
        wk_sb = sing.tile([128, KT, FPC], MM)
        wv_sb = sing.tile([128, KT, FPC], MM)
        nc.sync.dma_start(wq_sb, wqT[:, :].rearrange("(ko p) f -> p ko f", p=128))
        nc.sync.dma_start(wk_sb, wkT[:, :].rearrange("(ko p) f -> p ko f", p=128))
        nc.sync.dma_start(wv_sb, wvT[:, :].rearrange("(ko p) f -> p ko f", p=128))
        wo_sb = sing.tile([FPC, HID], MM)
        nc.sync.dma_start(wo_sb, woT[:, :])
        b_sb = sing.tile([FPC, 3], f32)
        nc.sync.dma_start(b_sb, bqkv[:, :])
        cos_sb = sing.tile([128, S], f32)
        sin_sb = sing.tile([128, S], f32)
        nc.sync.dma_start(cos_sb[0:64, :], cosT[:, :])
        nc.sync.dma_start(cos_sb[64:128, :], cosT[:, :])
        nc.sync.dma_start(sin_sb[0:64, :], sinTs[:, :])
        nc.sync.dma_start(sin_sb[64:128, :], sinTs[:, :])
        ones_sb = sing.tile([1, 64], f32)
        nc.vector.memset(ones_sb, 1.0)
        ident = sing.tile([128, 128], f32)
        make_identity(nc, ident)

        qT_sb = sing.tile([128, S], MM)
        kT_sb = sing.tile([128, S], MM)
        v1_sb = sing.tile([128, 2, SK, 65], MM)
        # ones column (denominator row of the PV matmul): DMA-broadcast from an
        # inline constant — memset can't write the matmul dtype directly
        ones_dram = nc.inline_tensor(np.ones((128, 1), dtype=np.float32), name="onecol")
        ones_bcast = bass.AP(
            tensor=ones_dram,
            offset=0,
            ap=[[1, 128], [0, 2 * SK], [1, 1]],
        )
        v1_flat = v1_sb.rearrange("p a b c -> p (a b) c")
        nc.gpsimd.dma_start(v1_flat[:, :, 64:65], ones_bcast)
        ctx_sb = sing.tile([128, S], MM)

        # ---- projections + RoPE + v transpose ----
        with ExitStack() as ph1:
            hp = ph1.enter_context(tc.tile_pool(name="hp", bufs=2))
            vt = ph1.enter_context(tc.tile_pool(name="vt", bufs=2))
            rp = ph1.enter_context(tc.tile_pool(name="rope", bufs=3))
            pps = ph1.enter_context(tc.tile_pool(name="pps", bufs=3, space="PSUM"))
            tps = ph1.enter_context(tc.tile_pool(name="tps", bufs=2, space="PSUM"))
            for ch in range(NPC):
                ssl = slice(ch * PC, (ch + 1) * PC)
                h_sb = hp.tile([128, KT, PC], MM)
                nc.sync.dma_start(h_sb, hT3[:, :, ssl])
                for wi, (w_sb, dst) in enumerate(
                    [(wq_sb, qT_sb), (wk_sb, kT_sb), (wv_sb, None)]
                ):
                    ps = pps.tile([128, PC], f32)
                    for k in range(KT):
                        nc.tensor.matmul(
                            ps,
                            w_sb[:, k, :],
                            h_sb[:, k, :],
                            start=(k == 0),
                            stop=(k == KT - 1),
                        )
                    if dst is not None:
                        # bias add -> f32 staging, then RoPE below writes MM dst
                        stg = rp.tile([128, PC], f32, tag="stg", name=f"stg_{ch}_{wi}")
                        nc.vector.tensor_scalar_add(stg, ps, b_sb[:, wi : wi + 1])
                        tmp = rp.tile([128, PC], f32, tag="tmp", name=f"tmp_{ch}_{wi}")
                        # sin table is permuted+sign-folded on host so that the
                        # factor for destination rows `da` sits at source rows
                        # `sa` (keeps both DVE inputs at the same base partition)
                        for (da, sa) in ((0, 32), (32, 0), (64, 96), (96, 64)):
                            nc.vector.tensor_mul(
                                tmp[da : da + 32, :],
                                stg[sa : sa + 32, :],
                                sin_sb[sa : sa + 32, ssl],
                            )
                        nc.vector.tensor_mul(stg, stg, cos_sb[:, ssl])
                        # final add casts into the MM-typed q/k tensor
                        nc.vector.tensor_add(dst[:, ssl], stg, tmp)
                    else:
                        vtmp = vt.tile([128, PC], f32)
                        nc.vector.tensor_scalar_add(vtmp, ps, b_sb[:, wi : wi + 1])
                        for st in range(PC // 128):
                            for hh in range(2):
                                tp = tps.tile([128, 64], f32)
                                nc.tensor.transpose(
                                    tp,
                                    vtmp[hh * 64 : hh * 64 + 64, st * 128 : st * 128 + 128],
                                    ident[hh * 64 : hh * 64 + 64, hh * 64 : hh * 64 + 64],
                                )
                                nc.vector.tensor_copy(
                                    v1_sb[:, hh, ch * (PC // 128) + st, 0:64], tp
                                )

        # ---- attention + o_proj ----
        with ExitStack() as ph2:
            sp = ph2.enter_context(tc.tile_pool(name="sp", bufs=2, space="PSUM"))
            cxp = ph2.enter_context(tc.tile_pool(name="cxp", bufs=2, space="PSUM"))
            msp = ph2.enter_context(tc.tile_pool(name="msp", bufs=2, space="PSUM"))
            ptp = ph2.enter_context(tc.tile_pool(name="ptp", bufs=8))
            mss = ph2.enter_context(tc.tile_pool(name="mss", bufs=2))
            osb = ph2.enter_context(tc.tile_pool(name="osb", bufs=3))
            for c in range(NCH):
                cs0 = c * CH
                for hh in range(2):
                    hsl = slice(hh * 64, hh * 64 + 64)
                    cx = [
                        cxp.tile([65, HF], f32, tag="cx", name=f"cx_{c}_{hh}_{z}")
                        for z in range(2)
                    ]
                    for i in range(SK):
                        ksl = slice(i * 128, (i + 1) * 128)
                        ss = sp.tile([128, CH], f32)
                        for z in range(2):
                            nc.tensor.matmul(
                                ss[:, z * HF : (z + 1) * HF],
                                kT_sb[hsl, ksl],
                                qT_sb[hsl, cs0 + z * HF : cs0 + (z + 1) * HF],
                                start=True,
                                stop=True,
                            )
                        pt = ptp.tile([128, CH], MM)
                        nc.scalar.activation(pt, ss, Exp, scale=0.125)
                        for z in range(2):
                            nc.tensor.matmul(
                                cx[z],
                                v1_sb[:, hh, i, :],
                                pt[:, z * HF : (z + 1) * HF],
                                start=(i == 0),
                                stop=(i == SK - 1),
                            )
                    den = mss.tile([1, CH], f32)
                    for z in range(2):
                        nc.vector.tensor_copy(den[:, z * HF : (z + 1) * HF], cx[z][64:65, :])
                    rec = mss.tile([1, CH], f32)
                    nc.vector.reciprocal(rec, den)
                    rb = mss.tile([64, CH], f32)
                    for z in range(2):
                        rp_ps = msp.tile([64, HF], f32, tag="mm", name=f"rp_{c}_{hh}_{z}")
                        # tiny K=1 broadcast matmul in plain fp32 (legal, cheap)
                        nc.tensor.matmul(
                            rp_ps,
                            ones_sb,
                            rec[:, z * HF : (z + 1) * HF],
                            start=True,
                            stop=True,
                        )
                        nc.vector.tensor_copy(rb[:, z * HF : (z + 1) * HF], rp_ps)
                    for z in range(2):
                        # normalize; cast into MM-typed ctx tensor
                        nc.vector.tensor_mul(
                            ctx_sb[hsl, cs0 + z * HF : cs0 + (z + 1) * HF],
                            cx[z][0:64, :],
                            rb[:, z * HF : (z + 1) * HF],
                        )
                # o_proj for this chunk (both heads' ctx ready)
                for sq in range(CH // 128):
                    r0 = cs0 + sq * 128
                    for nz in range(HID // 512):
                        ops = msp.tile([128, 512], f32, tag="mm", name=f"op_{c}_{sq}_{nz}")
                        nc.tensor.matmul(
                            ops,
                            ctx_sb[:, r0 : r0 + 128],
                            wo_sb[:, nz * 512 : (nz + 1) * 512],
                            start=True,
                            stop=True,
                        )
                        ob = osb.tile([128, 512], f32)
                        nc.vector.tensor_copy(ob, ops)
                        nc.sync.dma_start(
                            out[r0 : r0 + 128, nz * 512 : (nz + 1) * 512], ob
                        )
    return nc


def _legalize_sync_waits(nc, max_waits=1):
    """Cap sync waits per instruction for this container's walrus build.

    The bundled walrus encodes a limited number of sync-wait commands per
    instruction ("Too many sync wait commands" codegen error), while Tile
    attaches one wait per logical processor where needed. An attached wait
    is equivalent to a standalone preceding wait on the same engine (that
    is exactly what raw-bass `wait_ge` emits: a pure-wait
    InstEventSemaphore), so hoist the excess waits onto EventSemaphore
    instructions inserted right before the offender.
    """
    from concourse import mybir

    n_fixed = 0
    for fn in nc.m.functions:
        for b in fn.blocks:
            insts = b.instructions
            idx = 0
            while idx < len(insts):
                inst = insts[idx]
                si = inst.sync_info
                waits = list(si.on_wait) if si and si.on_wait else []
                if len(waits) > max_waits:
                    updates = list(si.on_update) if si and si.on_update else []
                    pre, keep = waits[: -max_waits], waits[-max_waits:]
                    clones = []
                    for j, w in enumerate(pre):
                        clones.append(
                            mybir.InstEventSemaphore(
                                name=f"{inst.name}_sw{j}",
                                engine=inst.engine,
                                ins=[],
                                outs=[],
                                sync_info=mybir.SyncInfo(on_wait=[w], on_update=[]),
                            )
                        )
                    inst.sync_info = mybir.SyncInfo(on_wait=keep, on_update=updates)
                    for j, clone in enumerate(clones):
                        insts.insert(idx + j, clone)
                        try:
                            nc.inst_map[clone.name] = clone
                        except Exception:
                            pass
                    idx += len(clones)
                    n_fixed += 1
                idx += 1
    return n_fixed


def get_nc(S=SEQ, mm_dt="float32r"):
    key = (S, mm_dt)
    if key not in _NC_CACHE:
        nc = _build_nc(S, mm_dt)
        _legalize_sync_waits(nc)
        _NC_CACHE[key] = nc
    return _NC_CACHE[key]


def _mm_np_dtype(mm_dt):
    if mm_dt == "bfloat16":
        import ml_dtypes

        return np.dtype(ml_dtypes.bfloat16)
    if mm_dt == "float16":
        return np.dtype(np.float16)
    return np.dtype(np.float32)  # float32r carries fp32 bits


def make_in_maps(h, cos, sin, wq, bq, wk, bk, wv, bv, wo, mm_dt="float32r"):
    """Host-side shard prep. h [B,S,HID] -> per-core input dict."""
    mdt = _mm_np_dtype(mm_dt)
    h = np.asarray(h, dtype=np.float32)
    S = h.shape[1]
    hT = np.ascontiguousarray(h[0].T).astype(mdt)  # [HID, S]
    cos = np.asarray(cos, dtype=np.float32)
    sin = np.asarray(sin, dtype=np.float32)
    cosT = np.ascontiguousarray(cos.T)  # [64, S]
    sinT = sin.T
    # rotate_half: q'[0:32] = q[:32]*cos - q[32:64]*sin[0:32]
    #              q'[32:64] = q[32:64]*cos + q[0:32]*sin[32:64]
    # The kernel computes tmp[da] = q[sa] * sinTs[sa] with (da,sa) row-halves
    # swapped, so the table carries the destination row's signed sin at the
    # source row: sinTs[0:32] = +sin[32:64].T, sinTs[32:64] = -sin[0:32].T.
    sinTs = np.ascontiguousarray(
        np.concatenate([sinT[HEAD_DIM // 2 :], -sinT[: HEAD_DIM // 2]], axis=0)
    )
    wq = np.asarray(wq, dtype=np.float32)
    wk = np.asarray(wk, dtype=np.float32)
    wv = np.asarray(wv, dtype=np.float32)
    wo = np.asarray(wo, dtype=np.float32)
    bq = np.asarray(bq, dtype=np.float32)
    bk = np.asarray(bk, dtype=np.float32)
    bv = np.asarray(bv, dtype=np.float32)
    in_maps = []
    for c in range(NCORES):
        fs = slice(c * FPC, (c + 1) * FPC)
        in_maps.append(
            {
                "hT": hT,
                "wqT": np.ascontiguousarray(wq[fs, :].T).astype(mdt),
                "wkT": np.ascontiguousarray(wk[fs, :].T).astype(mdt),
                "wvT": np.ascontiguousarray(wv[fs, :].T).astype(mdt),
                "bqkv": np.ascontiguousarray(
                    np.stack([bq[fs], bk[fs], bv[fs]], axis=1)
                ),
                "woT": np.ascontiguousarray(wo[:, fs].T).astype(mdt),
                "cosT": cosT,
                "sinTs": sinTs,
            }
        )
    return in_maps


MM_DT = "float16"


def kernel(h, mask, cos, sin, wq, bq, wk, bk, wv, bv, wo, bo, **_unused):
    # mask is all-ones per the problem spec; post-softmax where(mask==0) is a no-op.
    from concourse.bass_utils import run_bass_kernel_spmd

    h = np.asarray(h, dtype=np.float32)
    S = h.shape[1]
    nc = get_nc(S, MM_DT)
    in_maps = make_in_maps(h, cos, sin, wq, bq, wk, bk, wv, bv, wo, MM_DT)
    res = run_bass_kernel_spmd(nc, in_maps, core_ids=list(range(NCORES)))
    acc = np.zeros((S, HIDDEN), dtype=np.float32)
    for r in res.results:
        acc += r["out"]
    acc += np.asarray(bo, dtype=np.float32)[None, :]
    return acc[None].astype(np.float32)
